# revision 4
# baseline (speedup 1.0000x reference)
"""ExplaiNN (dense_cnn) Trainium2 Bass kernel, 8-core SPMD.

Pipeline per reference:
  conv1d(4->300 units, K=19) + BN1 + exp + maxpool(7) -> per-unit fc1 (83->100)
  + BN2 + relu -> per-unit fc2 (100->1) + BN3 + relu -> final linear (300->2).

Distribution: conv+pool batch-sharded (16 b/core, all units), then an AllToAll
(issued per batch-half so the first overlaps conv of the second half)
exchanges pooled features so fc1/fc2/final run unit-sharded (38 u/core, full
batch 128).  Final [128,2] partials are summed on host.

Key layout choices vs the reference math:
  * BN1 affine is folded into the conv weights (a1*w) plus a ones-row in the
    im2col carrying c1, so conv PSUM holds y = a1*conv+c1 directly.
  * maxpool runs on raw y (monotone), as DVE reduce_max straight out of PSUM
    (8 batch x 63 positions = 9 pool windows per matmul, bank-sized).
  * transpose to p-on-partitions is done on the PE (identity matmul), four
    [84,128] tiles per PSUM bank; exp is fused into the PSUM->SBUF copy on
    the scalar engine (plain Exp, no scale/bias needed after folding).
  * fc1 is computed "flipped": lhsT = pooled data per unit, rhs = folded w1,
    giving PSUM [batch, 100]; fc2 then reduces on DVE (mul + reduce_sum),
    costing no PE matmuls at all.
"""

import numpy as np
import ml_dtypes

B, N, L, K, C1 = 128, 300, 600, 19, 100
PS = 7
LC = 581          # conv outputs needed (l = 0..580; 83 pool windows)
LP = 83
NCLS = 2
EPS = 1e-5

NCORES = 8
BLOC = B // NCORES            # 16 batch per core in phase A
NPAD = 304                    # units padded to 8*38
ULOC = NPAD // NCORES         # 38 units per core in phase B
CK = 4 * K + 1                # 77 contraction rows: 76 im2col + ones row (c1)
UCHUNKS = [(0, 128), (128, 128), (256, 48)]   # (start, size) unit chunks
HB = 8                        # batch per half (A2A pipelining granularity)

_CACHE = {}


def _build_bass():
    import concourse.bass as bass
    import concourse.bacc as bacc
    import concourse.mybir as mybir
    import concourse.tile as tile

    f32, bf16 = mybir.dt.float32, mybir.dt.bfloat16

    # Bacc (not plain Bass): its finalize() runs the wait-splitting passes
    # (move_matmul_waits_to_ldweights / generate_event_semaphores) that keep
    # every TPB command within its single hardware sync-wait slot.
    nc = bacc.Bacc("TRN2")
    xim = nc.declare_dram_parameter("xim", [CK, BLOC * LC], bf16, isOutput=False)
    wconv = nc.declare_dram_parameter("wconv", [CK, NPAD], bf16, isOutput=False)
    ident = nc.declare_dram_parameter("ident", [128, 128], f32, isOutput=False)
    w1aug = nc.declare_dram_parameter("w1aug", [LP + 1, ULOC * C1], bf16, isOutput=False)
    w2rep = nc.declare_dram_parameter("w2rep", [128, ULOC * C1], bf16, isOutput=False)
    c3rep = nc.declare_dram_parameter("c3rep", [128, ULOC], f32, isOutput=False)
    fwrep = nc.declare_dram_parameter("fwrep", [128, NCLS * ULOC], bf16, isOutput=False)
    out_part = nc.declare_dram_parameter("out_part", [B, NCLS], f32, isOutput=True)

    with tile.TileContext(nc) as tc:
        with (
            tc.tile_pool(name="dram", bufs=1, space="DRAM") as dram_pool,
            tc.tile_pool(name="singles", bufs=1) as singles,
            tc.tile_pool(name="scratch", bufs=1, space="PSUM") as scratch_pool,
        ):
            # DRAM exchange buffers, one pair per batch-half.
            # p2p layout [dst/src, p(84), b(8), u(38)]
            p2p_in = [dram_pool.tile([NCORES, LP + 1, HB, ULOC], bf16,
                                     name=f"p2p_in{h}") for h in range(2)]
            p2p_out = [dram_pool.tile([NCORES, LP + 1, HB, ULOC], bf16,
                                      name=f"p2p_out{h}") for h in range(2)]

            xim_sb = singles.tile([CK, BLOC, LC], bf16)
            nc.sync.dma_start(out=xim_sb, in_=xim[:, :])
            wconv_sb = singles.tile([CK, NPAD], bf16)
            nc.sync.dma_start(out=wconv_sb, in_=wconv[:, :])
            ident_sb = singles.tile([128, 128], f32)
            nc.sync.dma_start(out=ident_sb, in_=ident[:, :])
            w1_sb = singles.tile([LP + 1, ULOC * C1], bf16)
            nc.sync.dma_start(out=w1_sb, in_=w1aug[:, :])
            w2_sb = singles.tile([128, ULOC, C1], bf16)
            nc.sync.dma_start(out=w2_sb, in_=w2rep[:, :])
            c3_sb = singles.tile([128, ULOC], f32)
            nc.sync.dma_start(out=c3_sb, in_=c3rep[:, :])
            fw_sb = singles.tile([128, NCLS, ULOC], bf16)
            nc.sync.dma_start(out=fw_sb, in_=fwrep[:, :])

            # pooled raw y per unit-chunk; col 83 stays 0 -> exp gives the
            # fc1 bias ones-row after transpose
            praw = []
            for ci, (u0, P) in enumerate(UCHUNKS):
                t = singles.tile([128, BLOC, LP + 1], f32, name=f"praw{ci}")
                nc.vector.memset(t[:, :, LP:LP + 1], 0.0)
                praw.append(t)

            # transposed+exp'd pooled features, [p, b(8), u(304)] per half
            poolT = [singles.tile([LP + 1, HB, NPAD], bf16, name=f"poolT{h}")
                     for h in range(2)]
            # exchanged features, [p, src(8), b(16), u(38)]
            pTall = singles.tile([LP + 1, NCORES, BLOC, ULOC], bf16)

            # PE matmuls (notably the LDW path) only accept one sync wait; a
            # dummy bf16 matmul reading a freshly-DMA'd tile absorbs its
            # semaphore so the first real matmul of a phase needs one.
            def absorb(tile_ap):
                s = scratch_pool.tile([2, 2], f32, name="dummy", tag="dummy")
                src = tile_ap.bitcast(bf16) if tile_ap.dtype != bf16 else tile_ap
                src = src[0:1, 0:2]
                nc.tensor.matmul(out=s, lhsT=src, rhs=src,
                                 start=True, stop=True)

            # ---------------- conv + maxpool + transpose + exchange --------
            # l-chunks: 9 of 63 cols (9 pool windows each) + 1 of 14 (2 wins)
            with (
                tc.tile_pool(name="psA", bufs=2, space="PSUM") as psA,
                tc.tile_pool(name="psB", bufs=1, space="PSUM") as psB,
                tc.tile_pool(name="psT", bufs=2, space="PSUM") as psT,
            ):
                absorb(wconv_sb[0:1, 0:2])
                absorb(ident_sb[0:1, 0:2])
                for h in range(2):
                    b0 = h * HB
                    for ci, (u0, P) in enumerate(UCHUNKS):
                        lhsT = wconv_sb[:, u0:u0 + P]
                        for pair in range(4):
                            # two 63-col chunks -> two PSUM banks, one reduce
                            ps = psA.tile([128, 2, 512], f32, name="ps",
                                          tag="ps")
                            for k in range(2):
                                l0 = (2 * pair + k) * 63
                                nc.tensor.matmul(
                                    out=ps[0:P, k, 0:504].rearrange(
                                        "p (b l) -> p b l", l=63),
                                    lhsT=lhsT,
                                    rhs=xim_sb[:, b0:b0 + HB, l0:l0 + 63],
                                    start=True, stop=True,
                                )
                            nc.vector.reduce_max(
                                out=praw[ci][0:P, b0:b0 + HB,
                                             18 * pair:18 * pair + 18]
                                .rearrange("p b (x w) -> p x b w", w=9),
                                in_=ps[0:P, :, 0:504].rearrange(
                                    "p x (b w e) -> p x b w e", w=9, e=7),
                                axis=mybir.AxisListType.X,
                            )
                        # chunk 8: l 504..567, windows 72..80
                        ps8 = psB.tile([128, HB, 63], f32, name="ps8",
                                       tag="psb")
                        nc.tensor.matmul(
                            out=ps8[0:P],
                            lhsT=lhsT,
                            rhs=xim_sb[:, b0:b0 + HB, 504:567],
                            start=True, stop=True,
                        )
                        nc.vector.reduce_max(
                            out=praw[ci][0:P, b0:b0 + HB, 72:81],
                            in_=ps8[0:P].rearrange(
                                "p b (w e) -> p b w e", e=7),
                            axis=mybir.AxisListType.X,
                        )
                        # tail chunk: l 567..581, 2 windows
                        psb = psB.tile([128, HB, 63], f32, name="psb",
                                       tag="psb")
                        nc.tensor.matmul(
                            out=psb[0:P, :, 0:14],
                            lhsT=lhsT,
                            rhs=xim_sb[:, b0:b0 + HB, 567:581],
                            start=True, stop=True,
                        )
                        nc.vector.reduce_max(
                            out=praw[ci][0:P, b0:b0 + HB, 81:83],
                            in_=psb[0:P, :, 0:14].rearrange(
                                "p b (w e) -> p b w e", e=7),
                            axis=mybir.AxisListType.X,
                        )

                    # transpose (PE) + exp-copy (Act) for this half
                    for ci, (u0, P) in enumerate(UCHUNKS):
                        for q in range(2):
                            pst = psT.tile([LP + 1, 4, 128], f32, name="pst",
                                           tag="pst")
                            for k in range(4):
                                b = b0 + 4 * q + k
                                nc.tensor.transpose(
                                    out=pst[:, k, 0:P],
                                    in_=praw[ci][0:P, b, :],
                                    identity=ident_sb[0:P, 0:P],
                                )
                            nc.scalar.activation(
                                out=poolT[h][:, 4 * q:4 * q + 4, u0:u0 + P],
                                in_=pst[:, :, 0:P],
                                func=mybir.ActivationFunctionType.Exp,
                            )
                    # stage + exchange this half
                    for j in range(NCORES):
                        nc.sync.dma_start(
                            out=p2p_in[h][j, :, :, :],
                            in_=poolT[h][:, :, j * ULOC:(j + 1) * ULOC],
                        )
                    nc.gpsimd.collective_compute(
                        "AllToAll",
                        mybir.AluOpType.bypass,
                        replica_groups=[list(range(NCORES))],
                        ins=[p2p_in[h][:]],
                        outs=[p2p_out[h][:]],
                    )
                    src = bass.AP(
                        tensor=p2p_out[h].tensor,
                        offset=0,
                        ap=[[HB * ULOC, LP + 1],
                            [(LP + 1) * HB * ULOC, NCORES],
                            [ULOC, HB], [1, ULOC]],
                    )
                    nc.sync.dma_start(
                        out=pTall[:, :, b0:b0 + HB, :], in_=src)

            # ---------------- fc1 (flipped: out [batch, 100] per unit) -----
            h2_sb = singles.tile([128, ULOC, C1], bf16)
            with tc.tile_pool(name="psF", bufs=2, space="PSUM") as psF:
                absorb(w1_sb[0:1, 0:2])
                ngroups = (ULOC + 3) // 4
                for g in range(ngroups):
                    un = min(4, ULOC - 4 * g)
                    psf = psF.tile([128, 4, C1], f32, name="psf", tag="psf")
                    for k in range(un):
                        u = 4 * g + k
                        nc.tensor.matmul(
                            out=psf[:, k, :],
                            lhsT=pTall[:, :, :, u],
                            rhs=w1_sb[:, u * C1:(u + 1) * C1],
                            start=True, stop=True,
                        )
                    dst = h2_sb[:, 4 * g:4 * g + un, :]
                    if g % 2 == 0:
                        nc.scalar.activation(
                            out=dst, in_=psf[:, 0:un, :],
                            func=mybir.ActivationFunctionType.Relu,
                        )
                    else:
                        nc.vector.tensor_scalar_max(
                            out=dst, in0=psf[:, 0:un, :], scalar1=0.0,
                        )

            # ---------------- fc2 on DVE (no PE) ---------------------------
            prod_sb = singles.tile([128, ULOC, C1], bf16)
            nc.vector.tensor_mul(out=prod_sb, in0=h2_sb, in1=w2_sb)
            fc2o = singles.tile([128, ULOC], f32)
            nc.vector.reduce_sum(out=fc2o, in_=prod_sb,
                                 axis=mybir.AxisListType.X)
            nc.vector.tensor_add(out=fc2o, in0=fc2o, in1=c3_sb)
            h3_sb = singles.tile([128, ULOC], bf16)
            nc.vector.tensor_scalar_max(out=h3_sb, in0=fc2o, scalar1=0.0)

            # ---------------- final linear (partial over my units) ---------
            out_sb = singles.tile([B, NCLS], f32)
            prod2 = singles.tile([B, ULOC], f32)
            for cls in range(NCLS):
                nc.vector.tensor_mul(out=prod2, in0=h3_sb, in1=fw_sb[:, cls, :])
                nc.vector.reduce_sum(
                    out=out_sb[:, cls:cls + 1], in_=prod2,
                    axis=mybir.AxisListType.X,
                )
            nc.sync.dma_start(out=out_part[:, :], in_=out_sb)

    nc.finalize()
    return nc


def _host_prep(inputs):
    """Fold BN affines into weights, build per-core input maps."""
    x = np.asarray(inputs["x"], np.float32)
    conv_w = np.asarray(inputs["conv_w"], np.float32)
    conv_b = np.asarray(inputs["conv_b"], np.float32)
    g1, b1 = np.asarray(inputs["bn1_g"], np.float32), np.asarray(inputs["bn1_b"], np.float32)
    m1, v1 = np.asarray(inputs["bn1_m"], np.float32), np.asarray(inputs["bn1_v"], np.float32)
    fc1_w, fc1_b = np.asarray(inputs["fc1_w"], np.float32), np.asarray(inputs["fc1_b"], np.float32)
    g2, b2 = np.asarray(inputs["bn2_g"], np.float32), np.asarray(inputs["bn2_b"], np.float32)
    m2, v2 = np.asarray(inputs["bn2_m"], np.float32), np.asarray(inputs["bn2_v"], np.float32)
    fc2_w, fc2_b = np.asarray(inputs["fc2_w"], np.float32), np.asarray(inputs["fc2_b"], np.float32)
    g3, b3 = np.asarray(inputs["bn3_g"], np.float32), np.asarray(inputs["bn3_b"], np.float32)
    m3, v3 = np.asarray(inputs["bn3_m"], np.float32), np.asarray(inputs["bn3_v"], np.float32)
    final_w = np.asarray(inputs["final_w"], np.float32)
    final_b = np.asarray(inputs["final_b"], np.float32)
    bf = ml_dtypes.bfloat16

    a1 = g1 / np.sqrt(v1 + EPS)                      # [300] > 0
    c1 = a1 * (conv_b - m1) + b1                     # [300]
    a2 = g2 / np.sqrt(v2 + EPS)                      # [300,100]
    c2 = b2 - a2 * m2 + a2 * fc1_b                   # [300,100]
    a3 = g3 / np.sqrt(v3 + EPS)                      # [300]
    c3 = a3 * (fc2_b - m3) + b3                      # [300]

    # conv weights [77, 304]: rows (c*19+k) = a1*w, row 76 = c1; cols pad 0
    wconv = np.zeros((CK, NPAD), np.float32)
    wconv[:CK - 1, :N] = (conv_w * a1[:, None, None]).transpose(1, 2, 0).reshape(CK - 1, N)
    wconv[CK - 1, :N] = c1

    # fc1: per unit [84, 100]; rows 0..82 = (a2*w1) p-major, row 83 = c2
    w1full = np.zeros((NPAD, LP + 1, C1), np.float32)
    w1full[:N, :LP, :] = (fc1_w * a2[:, :, None]).transpose(0, 2, 1)
    w1full[:N, LP, :] = c2

    w2full = np.zeros((NPAD, C1), np.float32)
    w2full[:N] = fc2_w * a3[:, None]
    c3full = np.zeros(NPAD, np.float32)
    c3full[:N] = c3
    fwfull = np.zeros((NCLS, NPAD), np.float32)
    fwfull[:, :N] = final_w

    ident = np.eye(128, dtype=np.float32)

    in_maps = []
    for i in range(NCORES):
        us = slice(i * ULOC, (i + 1) * ULOC)
        xl = x[i * BLOC:(i + 1) * BLOC]              # [16, 4, 600]
        sw = np.lib.stride_tricks.sliding_window_view(xl, LC, axis=2)
        # sw[b, c, k, l] = x[b, c, k+l]; rows (c,k), plus ones row
        im = np.empty((CK, BLOC, LC), np.float32)
        im[:CK - 1] = sw[:, :, :K, :].transpose(1, 2, 0, 3).reshape(CK - 1, BLOC, LC)
        im[CK - 1] = 1.0

        w1c = w1full[us].transpose(1, 0, 2).reshape(LP + 1, ULOC * C1)
        w2c = np.broadcast_to(w2full[us].reshape(1, ULOC * C1), (128, ULOC * C1))
        c3c = np.broadcast_to(c3full[us].reshape(1, ULOC), (128, ULOC))
        fwc = np.broadcast_to(fwfull[:, us].reshape(1, NCLS * ULOC),
                              (128, NCLS * ULOC))
        in_maps.append({
            "xim": np.ascontiguousarray(im.reshape(CK, BLOC * LC)).astype(bf),
            "wconv": wconv.astype(bf),
            "ident": ident,
            "w1aug": np.ascontiguousarray(w1c).astype(bf),
            "w2rep": np.ascontiguousarray(w2c).astype(bf),
            "c3rep": np.ascontiguousarray(c3c),
            "fwrep": np.ascontiguousarray(fwc).astype(bf),
        })
    return in_maps, final_b


def kernel(**inputs):
    from concourse.bass_utils import run_bass_kernel_spmd

    if "nc" not in _CACHE:
        _CACHE["nc"] = _build_bass()
    nc = _CACHE["nc"]

    in_maps, final_b = _host_prep(inputs)
    res = run_bass_kernel_spmd(nc, in_maps, core_ids=list(range(NCORES)))
    out = np.zeros((B, NCLS), np.float32)
    for r in res.results:
        out += r["out_part"]
    out += final_b[None, :]
    return out
